# revision 1
# baseline (speedup 1.0000x reference)
"""Self-contained Trainium2 Bass kernel for the CrossAttention problem.

Shapes (hardcoded): B=4, L=2048, D=512, H=8, Dh=64, PF=2048.
Sharding: 8 cores = 2 inputs (question/query) x 4 batches. No collectives --
each core computes K/V projections for BOTH inputs of its batch, builds the
shared linear-attention state S = K1^T V1 + K2^T V2 (identical for both
sides), applies it to its own Q, then out-proj + LN + FFN + LN.

All activations are kept feature-major ("transposed", [D, L]) so every GEMM
contraction lives on SBUF partitions. All matmuls run in float32r.
"""
import sys

for _p in ("/opt/trn_rl_repo", "/root/.axon_site/_ro/trn_rl_repo"):
    if _p not in sys.path:
        sys.path.insert(0, _p)

import numpy as np

import concourse.bass as bass
import concourse.bacc as bacc
import concourse.tile as tile
from concourse import bass_utils, mybir

B = 4
L = 2048
D = 512
H = 8
DH = 64
PF = 2048
P = 128
NS = 512            # l-slice width (psum bank / fp32 moving-operand max)
NSL = L // NS       # 4 slices
DC = D // P         # 4 feature chunks
PFC = PF // P       # 16 pf chunks
EPS = 1e-5
INV_D = 1.0 / D

f32 = mybir.dt.float32
f32r = mybir.dt.float32r
AF = mybir.ActivationFunctionType
OP = mybir.AluOpType

_CACHE = {}


def _ln_block(nc, tc, pools, zsl, consts, n_chunks=DC, inv_n=INV_D):
    """LayerNorm (transposed layout) on one l-slice.

    zsl(d) -> AP of the [P, NS] chunk-d slice of z. Stats are computed with
    ones-matmuls (which broadcast the column sums to all partitions), then
    y = g*(z - mu)*rstd + b is applied IN PLACE into z via:
        t1 = z * rstdB           (DVE)
        t1 = Identity(t1*g + b)  (ACT, per-partition scale/bias)
        y  = t1 - u              (DVE; u = (g/N)*(S1*rstd), all on DVE since
                                  S1 is already partition-broadcast)
    """
    ptmp, ps_st = pools
    ones_t, g4, lnb4, eps_t = consts

    st_ps = ps_st.tile([P, 2, NS], f32, tag="stps", name="st_ps")
    for d in range(n_chunks):
        nc.tensor.matmul(st_ps[:, 0, :], ones_t, zsl(d),
                         start=(d == 0), stop=(d == n_chunks - 1))
    for d in range(n_chunks):
        zq = ptmp.tile([P, NS], f32r, tag="zq", name="zq")
        nc.scalar.activation(out=zq, in_=zsl(d), func=AF.Square)
        nc.tensor.matmul(st_ps[:, 1, :], ones_t, zq,
                         start=(d == 0), stop=(d == n_chunks - 1))

    mu2 = ptmp.tile([P, NS], f32, tag="mu2", name="mu2")
    nc.scalar.activation(out=mu2, in_=st_ps[:, 0, :], func=AF.Square,
                         scale=inv_n)
    vt = ptmp.tile([P, NS], f32, tag="vt", name="vt")
    nc.vector.tensor_scalar(out=vt, in0=st_ps[:, 1, :], scalar1=inv_n,
                            scalar2=None, op0=OP.mult)
    nc.vector.tensor_tensor(out=vt, in0=vt, in1=mu2, op=OP.subtract)
    nc.scalar.activation(out=vt, in_=vt, func=AF.Sqrt, bias=eps_t)
    rstd = ptmp.tile([P, NS], f32, tag="rstd", name="rstd")
    nc.vector.reciprocal(out=rstd, in_=vt)
    # st_ps row 0 is already partition-broadcast (ones-matmul), so the
    # g*mu*rstd term needs no PE rank-1 matmul: m_full = S1*rstd on DVE,
    # then a fused (m_full * g) * (1/D) per-partition tensor_scalar.
    m_full = ptmp.tile([P, NS], f32, tag="m_r", name="m_full")
    nc.vector.tensor_tensor(out=m_full, in0=st_ps[:, 0, :], in1=rstd,
                            op=OP.mult)
    for d in range(n_chunks):
        u = ptmp.tile([P, NS], f32, tag="u", name="u")
        nc.vector.tensor_scalar(out=u, in0=m_full, scalar1=g4[:, d:d + 1],
                                scalar2=inv_n, op0=OP.mult, op1=OP.mult)
        t1 = ptmp.tile([P, NS], f32, tag="t1", name="t1")
        nc.vector.tensor_tensor(out=t1, in0=zsl(d), in1=rstd, op=OP.mult)
        nc.scalar.activation(out=t1, in_=t1, func=AF.Identity,
                             bias=lnb4[:, d:d + 1], scale=g4[:, d:d + 1])
        nc.vector.tensor_tensor(out=zsl(d), in0=t1, in1=u, op=OP.subtract)


def _build_nc(dbg=False, repeat=1):
    nc = bacc.Bacc("TRN2", target_bir_lowering=False, debug=False,
                   num_devices=8)

    def din(name, shape):
        return nc.dram_tensor(name, shape, f32r, kind="ExternalInput").ap()

    x_own = din("x_own", [D, L])
    x_oth = din("x_oth", [D, L])
    wqT = din("wqT", [D, D])
    wkT = din("wkT", [D, D])
    wvT = din("wvT", [D, D])
    woT = din("woT", [D, D])
    w1T = din("w1T", [D, PF])
    w2T = din("w2T", [PF, D])
    def dinf(name, shape):
        return nc.dram_tensor(name, shape, f32, kind="ExternalInput").ap()

    bq_d = dinf("bq", [D])
    bk_d = dinf("bk", [D])
    bv_d = dinf("bv", [D])
    bo_d = dinf("bo", [D])
    b1_d = dinf("b1", [PF])
    b2_d = dinf("b2", [D])
    lng_d = dinf("lng", [D])     # ln_g
    lnb_d = dinf("lnb", [D])     # ln_b
    lngs_d = din("lngs", [D])    # ln_g / D (feeds matmul -> f32r)
    yT = nc.dram_tensor("yT", [D, L], f32r, kind="ExternalOutput").ap()
    if dbg:
        dQ = nc.dram_tensor("dQ", [D, L], f32r, kind="ExternalOutput").ap()
        dS = nc.dram_tensor("dS", [P, DC * P], f32r,
                            kind="ExternalOutput").ap()
        dA = nc.dram_tensor("dA", [D, L], f32r, kind="ExternalOutput").ap()
        dY1 = nc.dram_tensor("dY1", [D, L], f32r, kind="ExternalOutput").ap()

    def bcast_row(v, n):
        # [n] dram vector -> [P, n] all partitions identical
        return bass.AP(tensor=v.tensor, offset=v.offset, ap=[[0, P], [1, n]])

    def chunked_col(v, nch):
        # [nch*P] dram vector -> [P, nch] (chunk c in column c)
        return bass.AP(tensor=v.tensor, offset=v.offset,
                       ap=[[1, P], [P, nch]])

    with tile.TileContext(nc) as tc:
      import contextlib
      rep_ctx = (tc.For_i(0, repeat, 1) if repeat > 1
                 else contextlib.nullcontext())
      with rep_ctx:
        gp = tc.alloc_tile_pool(name="gp", bufs=1)
        # ---- persistent tiles -------------------------------------------
        w1_t = gp.tile([P, DC, PF], f32r, name="w1_t")       # 32KB/part
        z_t = gp.tile([P, DC, L], f32r, name="z_t")          # 32KB (z / y1)
        s_sb = gp.tile([64, H, DH], f32, name="s_sb")        # 2KB
        s_blk = gp.tile([P, DC, P], f32, name="s_blk")       # 2KB
        s_blk_r = gp.tile([P, DC, P], f32r, name="s_blk_r")  # 2KB
        ones_f = gp.tile([P, P], f32, name="ones_f")
        ones_t = gp.tile([P, P], f32r, name="ones_t")
        eps_t = gp.tile([P, 1], f32, name="eps_t")
        bq4 = gp.tile([P, DC], f32, name="bq4")
        bo4 = gp.tile([P, DC], f32, name="bo4")
        b24 = gp.tile([P, DC], f32, name="b24")
        g4 = gp.tile([P, DC], f32, name="g4")
        lnb4 = gp.tile([P, DC], f32, name="lnb4")
        b116 = gp.tile([P, PFC], f32, name="b116")

        nc.sync.dma_start(out=bq4, in_=chunked_col(bq_d, DC))
        nc.sync.dma_start(out=bo4, in_=chunked_col(bo_d, DC))
        nc.sync.dma_start(out=b24, in_=chunked_col(b2_d, DC))
        nc.sync.dma_start(out=g4, in_=chunked_col(lng_d, DC))
        nc.sync.dma_start(out=lnb4, in_=chunked_col(lnb_d, DC))
        nc.sync.dma_start(out=b116, in_=chunked_col(b1_d, PFC))
        nc.vector.memset(ones_f, 1.0)
        nc.vector.tensor_copy(out=ones_t, in_=ones_f)
        nc.vector.memset(eps_t, EPS)

        ln_consts = (ones_t, g4, lnb4, eps_t)

        # qt_t spans stages A+B only; own pool released after stage B
        pab = tc.alloc_tile_pool(name="pab", bufs=1)
        qt_t = pab.tile([P, DC, L], f32r, name="qt_t")       # 32KB

        # ================= Stage A: QKV projections + S ===================
        pa = tc.alloc_tile_pool(name="pa", bufs=1)
        pa2 = tc.alloc_tile_pool(name="pa2", bufs=2)
        ps_q = tc.alloc_tile_pool(name="ps_q", bufs=2, space="PSUM")
        ps_kv = tc.alloc_tile_pool(name="ps_kv", bufs=2, space="PSUM")
        ps_s = tc.alloc_tile_pool(name="ps_s", bufs=1, space="PSUM")

        wq_t = pa.tile([P, DC, D], f32r, name="wq_t")
        wk_t = pa.tile([P, DC, D], f32r, name="wk_t")
        wv_t = pa.tile([P, DC, D], f32r, name="wv_t")
        bkb = pa.tile([P, D], f32, name="bkb")
        bvb = pa.tile([P, D], f32, name="bvb")
        for d in range(DC):
            nc.sync.dma_start(out=wq_t[:, d, :], in_=wqT[d * P:(d + 1) * P, :])
            nc.sync.dma_start(out=wk_t[:, d, :], in_=wkT[d * P:(d + 1) * P, :])
            nc.sync.dma_start(out=wv_t[:, d, :], in_=wvT[d * P:(d + 1) * P, :])
        nc.sync.dma_start(out=bkb, in_=bcast_row(bk_d, D))
        nc.sync.dma_start(out=bvb, in_=bcast_row(bv_d, D))

        s_ps = ps_s.tile([64, H, DH], f32, name="s_ps")
        sctr = 0
        n_s_groups = NSL * 4 * 2  # slices * chunks-per-slice * inputs

        def emit_s_group(k_sb, v_sb):
            # S matmuls for one (input, l-chunk) group; delayed one group
            # behind the K/V GEMMs so PE never stalls on the DVE bias-adds.
            nonlocal sctr
            first, last = sctr == 0, sctr == n_s_groups - 1
            sctr += 1
            for h in range(H):
                # start=True clears the WHOLE psum bank, so only the very
                # first matmul into this bank may carry it; the other heads'
                # first writes rely on has_written=0 overwrite semantics
                # after that single clear.
                nc.tensor.matmul(
                    s_ps[:, h, :],
                    k_sb[:, h * DH:(h + 1) * DH],
                    v_sb[:, h * DH:(h + 1) * DH],
                    start=(first and h == 0), stop=last)

        pending = None
        xo_list = []
        for n in range(NSL):
            # xo slices stay resident (bufs=4) and are reused by the Q^T
            # GEMMs below -- avoids a second 4MB load of x_own.
            xo_s = pa2.tile([P, DC, NS], f32r, tag="xo", bufs=4, name="xo_s")
            xo_list.append(xo_s)
            xt_s = pa2.tile([P, DC, NS], f32r, tag="xt", name="xt_s")
            for d in range(DC):
                nc.sync.dma_start(out=xo_s[:, d, :],
                                  in_=x_own[d * P:(d + 1) * P,
                                            n * NS:(n + 1) * NS])
                nc.sync.dma_start(out=xt_s[:, d, :],
                                  in_=x_oth[d * P:(d + 1) * P,
                                            n * NS:(n + 1) * NS])
            # K/V (natural layout) + S accumulation, both inputs
            for jj in range(4):
                for xs in (xo_s, xt_s):
                    k_ps = ps_kv.tile([P, D], f32, tag="kps", bufs=3,
                                      name="k_ps")
                    v_ps = ps_kv.tile([P, D], f32, tag="vps", name="v_ps")
                    for d in range(DC):
                        nc.tensor.matmul(k_ps, xs[:, d, jj * P:(jj + 1) * P],
                                         wk_t[:, d, :],
                                         start=(d == 0), stop=(d == DC - 1))
                    for d in range(DC):
                        nc.tensor.matmul(v_ps, xs[:, d, jj * P:(jj + 1) * P],
                                         wv_t[:, d, :],
                                         start=(d == 0), stop=(d == DC - 1))
                    k_sb = pa2.tile([P, D], f32r, tag="ksb", bufs=3,
                                    name="k_sb")
                    v_sb = pa2.tile([P, D], f32r, tag="vsb", bufs=3,
                                    name="v_sb")
                    nc.vector.tensor_tensor(out=k_sb, in0=k_ps, in1=bkb,
                                            op=OP.add)
                    nc.vector.tensor_tensor(out=v_sb, in0=v_ps, in1=bvb,
                                            op=OP.add)
                    if pending is not None:
                        emit_s_group(*pending)
                    pending = (k_sb, v_sb)
        emit_s_group(*pending)
        nc.vector.tensor_copy(out=s_sb, in_=s_ps)
        # assemble block-diagonal S: head pair c -> [128, 128] block with
        # head 2c in the top-left 64x64 and head 2c+1 in the bottom-right.
        nc.vector.memset(s_blk, 0.0)
        for c in range(DC):
            nc.vector.tensor_copy(out=s_blk[0:64, c, 0:64],
                                  in_=s_sb[:, 2 * c, :])
            # odd head needs a partition shift (64..127): SBUF->SBUF DMA
            nc.sync.dma_start(out=s_blk[64:128, c, 64:128],
                              in_=s_sb[:, 2 * c + 1, :])
        nc.vector.tensor_copy(out=s_blk_r, in_=s_blk)
        # w1 is not needed until stage C; load it here so the opening K/V
        # matmuls are not queued behind 4MB of FFN weight traffic.
        for d in range(DC):
            nc.sync.dma_start(out=w1_t[:, d, :], in_=w1T[d * P:(d + 1) * P, :])
        # Q^T last in stage A: its 64 matmuls overlap the S assembly tail
        for n in range(NSL):
            xq_s = xo_list[n]
            for o in range(DC):
                q_ps = ps_q.tile([P, NS], f32, tag="qps", name="q_ps")
                for d in range(DC):
                    nc.tensor.matmul(q_ps, wq_t[:, d, o * P:(o + 1) * P],
                                     xq_s[:, d, :],
                                     start=(d == 0), stop=(d == DC - 1))
                nc.scalar.activation(out=qt_t[:, o, n * NS:(n + 1) * NS],
                                     in_=q_ps, func=AF.Identity,
                                     bias=bq4[:, o:o + 1])
        if dbg:
            nc.sync.dma_start(out=dS, in_=s_blk_r.rearrange("p c m -> p (c m)"))
            for c in range(DC):
                for n in range(NSL):
                    nc.sync.dma_start(
                        out=dQ[c * P:(c + 1) * P, n * NS:(n + 1) * NS],
                        in_=qt_t[:, c, n * NS:(n + 1) * NS])

        ps_s.release()
        ps_kv.release()
        ps_q.release()
        pa2.release()
        pa.release()

        # ================= Stage B: attn, out-proj, LN1 ===================
        pb = tc.alloc_tile_pool(name="pb", bufs=1)
        pb2 = tc.alloc_tile_pool(name="pb2", bufs=2)
        ps_a = tc.alloc_tile_pool(name="ps_a", bufs=2, space="PSUM")
        ps_o = tc.alloc_tile_pool(name="ps_o", bufs=2, space="PSUM")
        ps_st = tc.alloc_tile_pool(name="ps_st", bufs=1, space="PSUM")

        at_t = pb.tile([P, DC, L], f32r, name="at_t")
        wo_t = pb.tile([P, DC, D], f32r, name="wo_t")
        for d in range(DC):
            nc.sync.dma_start(out=wo_t[:, d, :], in_=woT[d * P:(d + 1) * P, :])

        # attnT = S^T @ Q^T, one block-diagonal matmul per head pair
        for s in range(NSL):
            for c in range(DC):
                a_ps = ps_a.tile([P, NS], f32, tag="aps", bufs=3,
                                 name="a_ps")
                nc.tensor.matmul(a_ps, s_blk_r[:, c, :],
                                 qt_t[:, c, s * NS:(s + 1) * NS],
                                 start=True, stop=True)
                nc.vector.tensor_copy(out=at_t[:, c, s * NS:(s + 1) * NS],
                                      in_=a_ps)

        # O^T = Wo @ attnT, z = x + O + bo; LN1(s-1) emitted behind O(s)
        # so its DVE/ACT chain overlaps PE matmuls instead of stalling them.
        for s in range(NSL):
            xr_s = pb2.tile([P, DC, NS], f32r, tag="xr", name="xr_s")
            for d in range(DC):
                nc.sync.dma_start(out=xr_s[:, d, :],
                                  in_=x_own[d * P:(d + 1) * P,
                                            s * NS:(s + 1) * NS])
            for o in range(DC):
                o_ps = ps_o.tile([P, NS], f32, tag="ops", bufs=3,
                                 name="o_ps")
                for c in range(DC):
                    nc.tensor.matmul(o_ps, wo_t[:, c, o * P:(o + 1) * P],
                                     at_t[:, c, s * NS:(s + 1) * NS],
                                     start=(c == 0), stop=(c == DC - 1))
                zs = z_t[:, o, s * NS:(s + 1) * NS]
                nc.scalar.activation(out=zs, in_=o_ps, func=AF.Identity,
                                     bias=bo4[:, o:o + 1])
                nc.vector.tensor_tensor(out=zs, in0=zs, in1=xr_s[:, o, :],
                                        op=OP.add)
            if s >= 1:
                _ln_block(nc, tc, (pb2, ps_st),
                          lambda d, s=s - 1: z_t[:, d, s * NS:(s + 1) * NS],
                          ln_consts)
        _ln_block(nc, tc, (pb2, ps_st),
                  lambda d: z_t[:, d, (NSL - 1) * NS:NSL * NS], ln_consts)
        if dbg:
            for c in range(DC):
                for n in range(NSL):
                    nc.sync.dma_start(
                        out=dA[c * P:(c + 1) * P, n * NS:(n + 1) * NS],
                        in_=at_t[:, c, n * NS:(n + 1) * NS])
                    nc.sync.dma_start(
                        out=dY1[c * P:(c + 1) * P, n * NS:(n + 1) * NS],
                        in_=z_t[:, c, n * NS:(n + 1) * NS])

        ps_st.release()
        ps_o.release()
        ps_a.release()
        pb2.release()
        pb.release()
        pab.release()

        # ================= Stage C: FFN + LN2 + output ====================
        pc = tc.alloc_tile_pool(name="pc", bufs=1)
        pc2 = tc.alloc_tile_pool(name="pc2", bufs=2)
        ps_h = tc.alloc_tile_pool(name="ps_h", bufs=2, space="PSUM")
        ps_st2 = tc.alloc_tile_pool(name="ps_st2", bufs=1, space="PSUM")

        w2_t = pc.tile([P, PFC, D], f32r, name="w2_t")
        for k in range(PFC):
            nc.sync.dma_start(out=w2_t[:, k, :], in_=w2T[k * P:(k + 1) * P, :])

        def emit_ln2_tail(z2, s):
            _ln_block(nc, tc, (pc2, ps_st2),
                      lambda d, z2=z2: z2[:, d, :], ln_consts)
            for c in range(DC):
                nc.sync.dma_start(out=yT[c * P:(c + 1) * P,
                                         s * NS:(s + 1) * NS],
                                  in_=z2[:, c, :])

        prev_tail = None
        for s in range(NSL):
            h1_sb = pc.tile([P, PFC, NS], f32r, tag="h1", bufs=1, name="h1_sb")
            for pf in range(PFC):
                h_ps = ps_h.tile([P, NS], f32, tag="hps", bufs=3,
                                 name="h_ps")
                for d in range(DC):
                    nc.tensor.matmul(h_ps, w1_t[:, d, pf * P:(pf + 1) * P],
                                     z_t[:, d, s * NS:(s + 1) * NS],
                                     start=(d == 0), stop=(d == DC - 1))
                nc.scalar.activation(out=h1_sb[:, pf, :], in_=h_ps,
                                     func=AF.Relu, bias=b116[:, pf:pf + 1])
            z2 = pc2.tile([P, DC, NS], f32r, tag="z2", name="z2")
            for o in range(DC):
                f_ps = ps_h.tile([P, NS], f32, tag="fps", bufs=3,
                                 name="f_ps")
                for k in range(PFC):
                    nc.tensor.matmul(f_ps, w2_t[:, k, o * P:(o + 1) * P],
                                     h1_sb[:, k, :],
                                     start=(k == 0), stop=(k == PFC - 1))
                nc.scalar.activation(out=z2[:, o, :], in_=f_ps,
                                     func=AF.Identity, bias=b24[:, o:o + 1])
                nc.vector.tensor_tensor(out=z2[:, o, :], in0=z2[:, o, :],
                                        in1=z_t[:, o, s * NS:(s + 1) * NS],
                                        op=OP.add)
            if prev_tail is not None:
                emit_ln2_tail(*prev_tail)
            prev_tail = (z2, s)
        emit_ln2_tail(*prev_tail)

        ps_st2.release()
        ps_h.release()
        pc2.release()
        pc.release()
        gp.release()

    nc.compile()
    return nc


def get_nc(dbg=False, repeat=1):
    key = f"nc_{dbg}_{repeat}"
    if key not in _CACHE:
        _CACHE[key] = _build_nc(dbg=dbg, repeat=repeat)
    return _CACHE[key]


def _host_prep(inputs):
    f = lambda a: np.ascontiguousarray(np.asarray(a), dtype=np.float32)
    shared = {
        "wqT": f(np.asarray(inputs["Wq"]).T),
        "wkT": f(np.asarray(inputs["Wk"]).T),
        "wvT": f(np.asarray(inputs["Wv"]).T),
        "woT": f(np.asarray(inputs["Wo"]).T),
        "w1T": f(np.asarray(inputs["W1"]).T),
        "w2T": f(np.asarray(inputs["W2"]).T),
        "bq": f(inputs["bq"]), "bk": f(inputs["bk"]),
        "bv": f(inputs["bv"]), "bo": f(inputs["bo"]),
        "b1": f(inputs["b1"]), "b2": f(inputs["b2"]),
        "lng": f(inputs["ln_g"]), "lnb": f(inputs["ln_b"]),
        "lngs": f(np.asarray(inputs["ln_g"], dtype=np.float64) / D),
    }
    question = np.asarray(inputs["question"], dtype=np.float32)
    query = np.asarray(inputs["query"], dtype=np.float32)
    srcs = (question, query)
    in_maps = []
    for idx in range(2):
        for b in range(B):
            in_maps.append({
                "x_own": f(srcs[idx][b].T),
                "x_oth": f(srcs[1 - idx][b].T),
                **shared,
            })
    return in_maps


def run_sharded(inputs, trace=False, dbg=False, repeat=1):
    nc = get_nc(dbg=dbg, repeat=repeat)
    in_maps = _host_prep(inputs)
    res = bass_utils.run_bass_kernel_spmd(nc, in_maps,
                                          core_ids=list(range(8)),
                                          trace=trace)
    out = np.empty((B, L, 2 * D), np.float32)
    for c in range(8):
        idx, b = divmod(c, B)
        out[b, :, idx * D:(idx + 1) * D] = res.results[c]["yT"].T
    return out, res


def kernel(**inputs):
    out, _ = run_sharded(inputs)
    return out


if __name__ == "__main__":
    # smoke build
    get_nc()
    print("build ok")

